# revision 13
# baseline (speedup 1.0000x reference)
"""AdaptiveGravityAttention on 8 TRN2 NeuronCores (Bass/Tile, SPMD).

Sharding: heads across cores (2 heads/core, all batches), token-parallel
final projection via per-batch AllToAll. All matmuls fp16/bf16 inputs with
fp32 PSUM accumulation.

Math notes:
- softmax rows are rebased by exp(+6.4*g[i]) so the gravity bias becomes an
  additive band term 0.1*g[i]*max(64-dist,0) that is zero outside |i-j|<64;
  causal masking rides the same term via a -1e9 entry (g>0 always).
- logits are bounded (~|6|+6.4*g<=40) so exp needs no running max; exp
  outputs stored bf16 (range), q/k/v/x/W stored fp16 (precision).
"""
import os
import sys

sys.path.insert(0, "/opt/trn_rl_repo")

import numpy as np

import concourse.bass as bass  # noqa: F401  (engine types referenced via nc)
import concourse.mybir as mybir
import concourse.tile as tile
from concourse import bacc
from concourse.masks import make_identity
from concourse.bass_utils import run_bass_kernel_spmd

B, T, C = 4, 2048, 1024
H, D = 16, 64
NC = 8
HP = H // NC          # heads per core = 2
ROWS = HP * D         # 128 q/k/v rows per core
TB = T                # tokens per batch
QCH = 512             # q-chunk width
NQC = TB // QCH       # 4
KT = 128              # k-tile
NKT = TB // KT        # 16
PCH = 512             # projection token chunk
NPC = TB // PCH       # 4
TOK = T // NC         # tokens per core per batch for out-proj = 256

F16 = mybir.dt.float16
BF16 = mybir.dt.bfloat16
F32 = mybir.dt.float32

_CACHED = {}
LAST_RESULT = None
DEBUG = False


def build_nc():
    nc = bacc.Bacc("TRN2", target_bir_lowering=False, num_devices=NC)

    # ---- dram parameters (per-core shards prepared on host) ----
    xT = nc.declare_dram_parameter("xT", [C, B * T], F16, isOutput=False)
    rope2 = nc.declare_dram_parameter("rope2", [128, T], F32, isOutput=False)
    rope2s = nc.declare_dram_parameter("rope2s", [128, T], F32, isOutput=False)
    Wq = nc.declare_dram_parameter("Wq", [C, ROWS], F16, isOutput=False)
    Wk = nc.declare_dram_parameter("Wk", [C, ROWS], F16, isOutput=False)
    Wv = nc.declare_dram_parameter("Wv", [C, ROWS], F16, isOutput=False)
    gw = nc.declare_dram_parameter("gw", [C, HP], F16, isOutput=False)
    gb = nc.declare_dram_parameter("gb", [HP, 1], F32, isOutput=False)
    Rdiag = nc.declare_dram_parameter("Rdiag", [128, 128], F32, isOutput=False)
    Roff = nc.declare_dram_parameter("Roff", [128, 128], F32, isOutput=False)
    Wproj = nc.declare_dram_parameter("Wproj", [C, C], F16, isOutput=False)
    out = nc.declare_dram_parameter("out", [B, 8, 128, TOK], F32, isOutput=True)
    if DEBUG:
        dbg_q = nc.declare_dram_parameter("dbg_q", [128, TB], F16, isOutput=True)
        dbg_k = nc.declare_dram_parameter("dbg_k", [128, TB], F16, isOutput=True)
        dbg_g = nc.declare_dram_parameter("dbg_g", [HP, TB], F32, isOutput=True)
        dbg_yl = nc.declare_dram_parameter("dbg_yl", [NC, ROWS, TOK], F16, isOutput=True)
        dbg_yt = nc.declare_dram_parameter("dbg_yt", [NC, ROWS, TOK], F16, isOutput=True)
        dbg_va = nc.declare_dram_parameter("dbg_va", [128, HP, NKT, D + 1], BF16, isOutput=True)
        dbg_yps = nc.declare_dram_parameter("dbg_yps", [HP, NQC, D + 1, QCH], F32, isOutput=True)
        dbg_rb = nc.declare_dram_parameter("dbg_rb", [HP, NQC, D, QCH], F32, isOutput=True)
        dbg_yl2 = nc.declare_dram_parameter("dbg_yl2", [NC, ROWS, TOK], F16, isOutput=True)

    # ---- internal DRAM ----
    g_scr = nc.dram_tensor("g_scr", [B, HP, TB], F32)
    r_scr = nc.dram_tensor("r_scr", [B, HP, NQC, QCH], F32)
    y_loc = nc.dram_tensor("y_loc", [B, NC, ROWS, TOK], F16)
    y_tok = nc.dram_tensor("y_tok", [B, NC, ROWS, TOK], F16)
    warm_in = nc.dram_tensor("warm_in", [1, 64], F32)
    warm_out = nc.dram_tensor("warm_out", [NC, 64], F32, addr_space="Shared")

    groups = [list(range(NC))]

    with tile.TileContext(nc) as tc:
        with (
            tc.tile_pool(name="const", bufs=1) as constp,
            tc.tile_pool(name="xt", bufs=2) as xtp,
            tc.tile_pool(name="qkv", bufs=2) as qkvp,
            tc.tile_pool(name="rope", bufs=3) as ropep,
            tc.tile_pool(name="att", bufs=4) as attp,
            tc.tile_pool(name="nb", bufs=4) as nbp,
            tc.tile_pool(name="yout", bufs=2) as youtp,
            tc.tile_pool(name="fin", bufs=2) as finp,
            tc.tile_pool(name="psum", bufs=2, space="PSUM") as psp,
        ):
            # ---- warmup collective: absorbs cross-core start skew ----
            wt = constp.tile([1, 64], F32, tag="warm")
            nc.sync.dma_start(out=wt[:], in_=rope2[0:1, 0:64])
            nc.sync.dma_start(out=warm_in[:], in_=wt[:])
            nc.gpsimd.collective_compute(
                "AllGather", mybir.AluOpType.bypass, replica_groups=groups,
                ins=[warm_in[:]], outs=[warm_out[:]],
            )

            # ---- constants ----
            wq_sb = constp.tile([128, 8, ROWS], F16, tag="wq")
            wk_sb = constp.tile([128, 8, ROWS], F16, tag="wk")
            wv_sb = constp.tile([128, 8, ROWS], F16, tag="wv")
            gw_sb = constp.tile([128, 8, HP], F16, tag="gw")
            nc.sync.dma_start(out=wq_sb[:], in_=Wq.rearrange("(t p) m -> p t m", p=128))
            nc.sync.dma_start(out=wk_sb[:], in_=Wk.rearrange("(t p) m -> p t m", p=128))
            nc.sync.dma_start(out=wv_sb[:], in_=Wv.rearrange("(t p) m -> p t m", p=128))
            nc.sync.dma_start(out=gw_sb[:], in_=gw.rearrange("(t p) m -> p t m", p=128))
            gb_sb = constp.tile([HP, 1], F32, tag="gb")
            nc.sync.dma_start(out=gb_sb[:], in_=gb[:])
            rd_sb = constp.tile([128, 128], F32, tag="rd")
            ro_sb = constp.tile([128, 128], F32, tag="ro")
            nc.sync.dma_start(out=rd_sb[:], in_=Rdiag[:])
            nc.sync.dma_start(out=ro_sb[:], in_=Roff[:])
            wp_sb = constp.tile([128, 8, C], F16, tag="wp")
            nc.sync.dma_start(out=wp_sb[:], in_=Wproj.rearrange("(t p) m -> p t m", p=128))
            ident = constp.tile([128, 64], F16, tag="id")
            make_identity(nc, ident[0:64, :])
            make_identity(nc, ident[64:128, :])

            # cos2/sin2 from rope inputs (ACT Sin; cos(x)=sin(x+pi/2))
            rp2 = constp.tile([128, T], F32, tag="rp2")
            rp2s = constp.tile([128, T], F32, tag="rp2s")
            nc.sync.dma_start(out=rp2[:], in_=rope2[:])
            nc.sync.dma_start(out=rp2s[:], in_=rope2s[:])
            # host pre-wraps phases into [-pi, pi] (ACT Sin LUT valid range):
            # rope2 holds wrap(pos + pi/2) so Sin gives cos(pos); rope2s holds
            # wrap(+-pos) so Sin gives the sign-baked sin term.
            cos2 = constp.tile([128, T], F32, tag="cos2")
            sin2 = constp.tile([128, T], F32, tag="sin2")
            nc.scalar.activation(cos2[:], rp2[:], mybir.ActivationFunctionType.Sin,
                                 bias=0.0, scale=1.0)
            nc.scalar.activation(sin2[:], rp2s[:], mybir.ActivationFunctionType.Sin,
                                 bias=0.0, scale=1.0)

            for b in range(B):
                # ================= projections for batch b =================
                qT = qkvp.tile([128, TB], F16, tag="qT")
                kT = qkvp.tile([128, TB], F16, tag="kT")
                vT = qkvp.tile([128, TB], F16, tag="vT")
                for ch in range(NPC):
                    tok0 = b * TB + ch * PCH
                    xt = xtp.tile([128, 8, PCH], F16, tag="xt")
                    nc.sync.dma_start(
                        out=xt[:],
                        in_=xT.rearrange("(t p) n -> p t n", p=128)[:, :, tok0:tok0 + PCH],
                    )
                    cs = slice(ch * PCH, (ch + 1) * PCH)

                    for which, w_sb in (("q", wq_sb), ("k", wk_sb)):
                        ps = psp.tile([128, PCH], F32, tag="mm")
                        for kt in range(8):
                            nc.tensor.matmul(ps[:], w_sb[:, kt, :], xt[:, kt, :],
                                             start=(kt == 0), stop=(kt == 7))
                        # rope: dst = ps*cos2 + swap(ps)*sin2
                        qf = ropep.tile([128, PCH], F32, tag="qf")
                        nc.vector.tensor_copy(qf[:], ps[:])
                        qsw = ropep.tile([128, PCH], F32, tag="qsw")
                        for blk in range(4):
                            srow = (blk ^ 1) * 32
                            nc.sync.dma_start(out=qsw[blk * 32:(blk + 1) * 32, :],
                                              in_=qf[srow:srow + 32, :])
                        t1 = ropep.tile([128, PCH], F32, tag="t1")
                        nc.vector.tensor_mul(t1[:], qf[:], cos2[:, cs])
                        t2 = ropep.tile([128, PCH], F32, tag="t2")
                        nc.vector.tensor_mul(t2[:], qsw[:], sin2[:, cs])
                        dst = qT if which == "q" else kT
                        nc.vector.tensor_add(dst[:, cs], t1[:], t2[:])

                    ps = psp.tile([128, PCH], F32, tag="mm")
                    for kt in range(8):
                        nc.tensor.matmul(ps[:], wv_sb[:, kt, :], xt[:, kt, :],
                                         start=(kt == 0), stop=(kt == 7))
                    nc.vector.tensor_copy(vT[:, cs], ps[:])

                    gps = psp.tile([HP, PCH], F32, tag="mm")
                    for kt in range(8):
                        nc.tensor.matmul(gps[:], gw_sb[:, kt, :], xt[:, kt, :],
                                         start=(kt == 0), stop=(kt == 7))
                    # softplus(z) = ln(1 + exp(z)); z in ~[-6,6] so no overflow
                    gex = ropep.tile([HP, PCH], F32, tag="gex")
                    nc.scalar.activation(gex[:], gps[:], mybir.ActivationFunctionType.Exp,
                                         bias=gb_sb[:], scale=1.0)
                    gsb = ropep.tile([HP, PCH], F32, tag="gsb")
                    nc.scalar.activation(gsb[:], gex[:], mybir.ActivationFunctionType.Ln,
                                         bias=1.0, scale=1.0)
                    nc.sync.dma_start(out=g_scr[b, :, ch * PCH:(ch + 1) * PCH], in_=gsb[:])

                # ====== V to token-major with ones column (PE transpose) ======
                vaug = qkvp.tile([128, HP, NKT, D + 1], BF16, tag="vaug")
                nc.vector.memset(vaug[:, :, :, D:D + 1], 1.0)
                for h in range(HP):
                    for tt in range(NKT):
                        tp = psp.tile([128, D], F16, tag="tp")
                        nc.tensor.transpose(
                            tp[:], vT[h * D:(h + 1) * D, tt * KT:(tt + 1) * KT],
                            ident[h * D:(h + 1) * D, :])
                        nc.vector.tensor_copy(vaug[:, h, tt, 0:D], tp[:])

                # ================= attention for batch b =================
                for h in range(HP):
                    hr = slice(h * D, (h + 1) * D)
                    for qc in range(NQC):
                        yps = psp.tile([D + 1, QCH], F32, tag="y")
                        njt = 4 * qc + 4  # k-tiles 0 .. 4qc+3
                        pend = None  # software pipeline: issue PV one step late
                        for j0 in range(njt):
                            r = j0 - 4 * qc
                            width = QCH if r < 0 else (4 - r) * KT
                            loc0 = max(r, 0) * KT
                            sps = psp.tile([128, QCH], F32, tag="s")
                            nc.tensor.matmul(
                                sps[:, 0:width],
                                kT[hr, j0 * KT:(j0 + 1) * KT],
                                qT[hr, qc * QCH + loc0:(qc + 1) * QCH],
                                start=True, stop=True)
                            # near-band additive gravity term (+ causal mask)
                            for kind, rt, i0 in (
                                ("diag", rd_sb, j0 * KT) if r >= 0 else (None, None, 0),
                                ("off", ro_sb, (j0 + 1) * KT) if -1 <= r <= 2 else (None, None, 0),
                            ):
                                if kind is None:
                                    continue
                                lc = 0 if (kind == "diag" or r < 0) else KT
                                gbc = nbp.tile([128, 128], F32, tag="gbc")
                                nc.sync.dma_start(
                                    out=gbc[:],
                                    in_=g_scr[b, h, i0:i0 + 128].unsqueeze(0).to_broadcast((128, 128)))
                                tmp = nbp.tile([128, 128], F32, tag="nbt")
                                nc.vector.tensor_mul(tmp[:], gbc[:], rt[:])
                                nc.vector.tensor_add(sps[:, lc:lc + 128], sps[:, lc:lc + 128], tmp[:])
                            et = attp.tile([128, QCH], BF16, tag="et")
                            nc.scalar.activation(et[:, 0:width], sps[:, 0:width],
                                                 mybir.ActivationFunctionType.Exp)
                            if pend is not None:
                                nc.tensor.matmul(*pend[0], start=pend[1], stop=False)
                            pend = ((yps[:, loc0:QCH], vaug[:, h, j0, :], et[:, 0:width]),
                                    j0 == 0)
                        nc.tensor.matmul(*pend[0], start=pend[1], stop=True)

                        # normalize: r = 1/rowsum; bounce via DRAM to broadcast
                        rsc = youtp.tile([128, QCH], F32, tag="rsc")
                        nc.vector.reciprocal(rsc[D:D + 1, :], yps[D:D + 1, :])
                        nc.sync.dma_start(out=r_scr[b, h, qc, :], in_=rsc[D:D + 1, :])
                        rb = youtp.tile([D, QCH], F32, tag="rb")
                        nc.sync.dma_start(
                            out=rb[:],
                            in_=r_scr[b, h, qc, :].unsqueeze(0).to_broadcast((D, QCH)))
                        if DEBUG and b == 0:
                            ydbg = youtp.tile([D + 1, QCH], F32, tag="ydbg")
                            nc.vector.tensor_copy(ydbg[:], yps[:])
                            nc.sync.dma_start(out=dbg_yps[h, qc], in_=ydbg[:])
                            nc.sync.dma_start(out=dbg_rb[h, qc], in_=rb[:])
                        yn = youtp.tile([D, QCH], F16, tag="yn")
                        nc.vector.tensor_mul(yn[:], yps[0:D, :], rb[:])
                        # scatter into y_loc shards (2 shards of 256 tokens per
                        # qchunk); plain slices — a partition-displacing
                        # rearrange on an SBUF source breaks dep tracking
                        for s in range(2):
                            nc.sync.dma_start(
                                out=y_loc[b, 2 * qc + s, hr, :],
                                in_=yn[:, s * TOK:(s + 1) * TOK])

                if DEBUG and b == 0:
                    nc.sync.dma_start(out=dbg_q[:], in_=qT[:])
                    nc.sync.dma_start(out=dbg_k[:], in_=kT[:])
                    nc.sync.dma_start(out=dbg_g[:], in_=g_scr[0])
                    nc.sync.dma_start(out=dbg_yl[:], in_=y_loc[0])
                    nc.sync.dma_start(out=dbg_va[:], in_=vaug[:])
                # redistribute heads->tokens for batch b
                nc.gpsimd.collective_compute(
                    "AllToAll", mybir.AluOpType.bypass, replica_groups=groups,
                    ins=[y_loc[b]], outs=[y_tok[b]],
                )

            if DEBUG:
                nc.sync.dma_start(out=dbg_yt[:], in_=y_tok[0])
                nc.sync.dma_start(out=dbg_yl2[:], in_=y_loc[0])
            # ================= output projection (my 256 tokens per batch) ====
            for b in range(B):
                yt = finp.tile([128, 8, TOK], F16, tag="yt")
                nc.sync.dma_start(
                    out=yt[:],
                    in_=y_tok[b].rearrange("s p n -> p s n"))
                for co in range(8):
                    fps = psp.tile([128, TOK], F32, tag="mm")
                    for kt in range(8):
                        nc.tensor.matmul(fps[:], wp_sb[:, kt, co * 128:(co + 1) * 128],
                                         yt[:, kt, :], start=(kt == 0), stop=(kt == 7))
                    ot = finp.tile([128, TOK], F32, tag="ot")
                    nc.vector.tensor_copy(ot[:], fps[:])
                    nc.sync.dma_start(out=out[b, co], in_=ot[:])

    nc.compile()
    nc.finalize()
    return nc


def _host_prep(x, rope, W_attn, W_proj, g_w, g_b):
    xT = np.ascontiguousarray(x.reshape(B * T, C).T).astype(np.float16)
    ropeT = np.ascontiguousarray(rope.T.astype(np.float32))  # [64, T]
    rope2 = np.concatenate([ropeT, ropeT], axis=0)  # [128, T]
    rope2s = rope2.copy()
    rr = np.arange(128)
    rope2s[(rr % 64) < 32] *= -1.0

    def wrap(a):  # into [-pi, pi] for the ACT Sin LUT
        return ((a + np.pi) % (2 * np.pi) - np.pi).astype(np.float32)

    rope2 = wrap(rope2 + np.pi / 2)   # Sin(rope2) == cos(pos)
    rope2s = wrap(rope2s)             # Sin(rope2s) == sign-baked sin(pos)
    Wproj16 = W_proj.astype(np.float16)

    idx = np.arange(128)
    ii, jj = idx[None, :], idx[:, None]  # Rdiag[j, i]
    rdiag = np.where(ii >= jj, 0.1 * np.maximum(64.0 - (ii - jj), 0.0), -1e9).astype(np.float32)
    roff = (0.1 * np.maximum(jj - ii - 64.0, 0.0)).astype(np.float32)

    in_maps = []
    for c in range(NC):
        h0 = HP * c
        cols = slice(h0 * D, (h0 + HP) * D)
        in_maps.append({
            "xT": xT,
            "rope2": rope2,
            "rope2s": rope2s,
            "Wq": np.ascontiguousarray(W_attn[:, cols] * 0.125).astype(np.float16),
            "Wk": np.ascontiguousarray(W_attn[:, C:][:, cols]).astype(np.float16),
            "Wv": np.ascontiguousarray(W_attn[:, 2 * C:][:, cols]).astype(np.float16),
            "gw": np.ascontiguousarray(g_w[:, h0:h0 + HP]).astype(np.float16),
            "gb": np.ascontiguousarray(g_b[h0:h0 + HP].reshape(HP, 1)).astype(np.float32),
            "Rdiag": rdiag,
            "Roff": roff,
            "Wproj": Wproj16,
        })
    return in_maps


def kernel(x, rope, W_attn, W_proj, g_w, g_b):
    global LAST_RESULT
    x = np.asarray(x, dtype=np.float32)
    rope = np.asarray(rope, dtype=np.float32)
    W_attn = np.asarray(W_attn, dtype=np.float32)
    W_proj = np.asarray(W_proj, dtype=np.float32)
    g_w = np.asarray(g_w, dtype=np.float32)
    g_b = np.asarray(g_b, dtype=np.float32)

    if "nc" not in _CACHED:
        _CACHED["nc"] = build_nc()
    nc = _CACHED["nc"]
    in_maps = _host_prep(x, rope, W_attn, W_proj, g_w, g_b)
    res = run_bass_kernel_spmd(nc, in_maps, list(range(NC)),
                               trace=bool(os.environ.get("AGA_TRACE")))
    LAST_RESULT = res
    # assemble: core c holds out[b, co, p, t] for tokens c*256..(c+1)*256 of each b
    full = np.empty((B, T, C), dtype=np.float32)
    for c in range(NC):
        oc = res.results[c]["out"]  # [B, 8, 128, TOK]
        for b in range(B):
            full[b, c * TOK:(c + 1) * TOK, :] = oc[b].reshape(C, TOK).T
    return full
